# revision 8
# baseline (speedup 1.0000x reference)
"""Trainium2 Bass kernel for masked general attention (ragged sequences).

reference computation per batch b:
    q       = query[b] @ W_in.T                      [Lq, D]
    S       = q @ context[b].T                       [Lq, Lk]
    S_m     = where(qmask & kmask, S, -1e9)
    W       = softmax(S_m, axis=-1)
    mix     = W @ context[b]                         [Lq, D]
    out     = tanh(concat([mix, q]) @ W_out.T)       [Lq, D]
    returns (out, S_m)

Sharding: data-parallel over batch. 32 batches / 8 cores = 4 per core,
same program on every core (SPMD), weights replicated.

Per-core schedule: for each batch, 4 q-blocks of 256 queries flow through
  proj (fp32r) -> scores (fp32r) -> min-mask -> softmax -> PE transpose
  -> mix (bf16) -> output matmul (bf16) -> tanh.
Masking uses elementwise min against +BIG/-1e9 vectors, which reproduces
the reference's exact -1e9 fill (scores are |S| < ~1e4 << 1e9) including
the uniform-softmax rows for fully-masked queries.
"""

import sys

sys.path.insert(0, "/opt/trn_rl_repo")

import numpy as np
import ml_dtypes

import concourse.bass as bass
import concourse.tile as tile
from concourse import bacc, mybir
from concourse import bass_utils
from concourse.masks import make_identity

F32 = mybir.dt.float32
F32R = mybir.dt.float32r
BF16 = mybir.dt.float16

B, Lq, Lk, D = 32, 1024, 1024, 1024
N_CORES = 8
BPC = B // N_CORES          # batches per core
MQ = 256                    # queries per block
NBLK = Lq // MQ             # q-blocks per batch
NEG = -1e9
BIG = 3.0e38

_program_cache = {}


def _build_program():
    nc = bacc.Bacc("TRN2", target_bir_lowering=False, debug=False,
                   num_devices=N_CORES)

    # DRAM I/O (per core shard). float32r tensors take np.float32 data.
    qT_d = nc.dram_tensor("qT", [BPC, D, Lq], F32R, kind="ExternalInput").ap()
    cT_d = nc.dram_tensor("cT", [BPC, D, Lk], F32R, kind="ExternalInput").ap()
    cn_d = nc.dram_tensor("cn", [BPC, Lk, D], BF16, kind="ExternalInput").ap()
    winT_d = nc.dram_tensor("winT", [D, D], F32R, kind="ExternalInput").ap()
    woT_d = nc.dram_tensor("woT", [2 * D, D], BF16, kind="ExternalInput").ap()
    kmin_d = nc.dram_tensor("kmin", [BPC, 128, Lk], F32, kind="ExternalInput").ap()
    qmin_d = nc.dram_tensor("qmin", [BPC, 128, Lq // 128], F32, kind="ExternalInput").ap()

    out_d = nc.dram_tensor("out", [BPC, Lq, D], F32, kind="ExternalOutput").ap()
    sc_d = nc.dram_tensor("sc", [BPC, Lq, Lk], F32, kind="ExternalOutput").ap()

    with tile.TileContext(nc) as tc:
        with (
            tc.tile_pool(name="static", bufs=1) as st,
            tc.tile_pool(name="ctx", bufs=1) as ctx_pool,
            tc.tile_pool(name="blk", bufs=2) as blk,
            tc.tile_pool(name="sm", bufs=4) as sm_pool,
            tc.tile_pool(name="ot", bufs=4) as ot_pool,
            tc.tile_pool(name="stats", bufs=4) as stats_pool,
            tc.tile_pool(name="psA", bufs=2, space="PSUM") as psA,
            tc.tile_pool(name="psS", bufs=2, space="PSUM") as psS,
            tc.tile_pool(name="psO", bufs=2, space="PSUM") as psO,
            tc.tile_pool(name="psT", bufs=2, space="PSUM") as psT,
        ):
            # ---- static weights -------------------------------------------
            winT_sb = st.tile([128, 8 * D], F32R, tag="winT")
            for dt in range(8):
                nc.sync.dma_start(winT_sb[:, dt * D:(dt + 1) * D],
                                  winT_d[dt * 128:(dt + 1) * 128, :])
            woT_sb = st.tile([128, 16 * D], BF16, tag="woT")
            for ct in range(16):
                nc.sync.dma_start(woT_sb[:, ct * D:(ct + 1) * D],
                                  woT_d[ct * 128:(ct + 1) * 128, :])
            ident = st.tile([128, 128], BF16, tag="ident")
            make_identity(nc, ident[:])

            for b in range(BPC):
                # ---- per-batch context + masks ----------------------------
                cT_sb = ctx_pool.tile([128, 8 * Lk], F32R, tag="cT")
                for et in range(8):
                    nc.sync.dma_start(cT_sb[:, et * Lk:(et + 1) * Lk],
                                      cT_d[b, et * 128:(et + 1) * 128, :])
                cn_sb = ctx_pool.tile([128, 8 * D], BF16, tag="cn")
                for kt in range(8):
                    nc.sync.dma_start(cn_sb[:, kt * D:(kt + 1) * D],
                                      cn_d[b, kt * 128:(kt + 1) * 128, :])
                kmin_sb = ctx_pool.tile([128, Lk], F32, tag="kmin")
                nc.sync.dma_start(kmin_sb[:], kmin_d[b])
                qmin_sb = ctx_pool.tile([128, Lq // 128], F32, tag="qmin")
                nc.sync.dma_start(qmin_sb[:], qmin_d[b])

                for blk_i in range(NBLK):
                    q0 = blk_i * MQ

                    # ---- stage 1: qT = W_in @ query (both transposed) -----
                    qry_sb = blk.tile([128, 8 * MQ], F32R, tag="qry")
                    for dt in range(8):
                        nc.sync.dma_start(
                            qry_sb[:, dt * MQ:(dt + 1) * MQ],
                            qT_d[b, dt * 128:(dt + 1) * 128, q0:q0 + MQ])
                    qTr_sb = blk.tile([128, 8 * MQ], F32R, tag="qm")
                    qTb_sb = blk.tile([128, 8 * MQ], BF16, tag="qTb")
                    for et in range(8):
                        pq = psA.tile([128, MQ], F32, tag="psA")
                        for dt in range(8):
                            nc.tensor.matmul(
                                pq[:],
                                winT_sb[:, dt * D + et * 128:dt * D + (et + 1) * 128],
                                qry_sb[:, dt * MQ:(dt + 1) * MQ],
                                start=(dt == 0), stop=(dt == 7))
                        nc.vector.tensor_copy(qTr_sb[:, et * MQ:(et + 1) * MQ], pq[:])
                        nc.scalar.copy(qTb_sb[:, et * MQ:(et + 1) * MQ], pq[:])

                    # ---- stage 2: scores + mask + softmax -----------------
                    ew_sb = blk.tile([128, 2 * Lk], BF16, tag="ew")
                    wt_sb = blk.tile([128, 8 * MQ], BF16, tag="wt")
                    for h in range(2):
                        jt = blk_i * 2 + h          # global q-tile index
                        rows = slice(q0 + h * 128, q0 + (h + 1) * 128)
                        stt = stats_pool.tile([128, 8], F32, tag="stats")
                        sm_n = []
                        for n in range(2):
                            ps = psS.tile([128, 512], F32, tag="psS")
                            for et in range(8):
                                nc.tensor.matmul(
                                    ps[:],
                                    qTr_sb[:, et * MQ + h * 128:et * MQ + (h + 1) * 128],
                                    cT_sb[:, et * Lk + n * 512:et * Lk + (n + 1) * 512],
                                    start=(et == 0), stop=(et == 7))
                            # k-mask then q-mask (exact -1e9 fill via min)
                            sm = sm_pool.tile([128, 512], F32, tag="sm")
                            sm_n.append(sm)
                            nc.vector.tensor_tensor(
                                sm[:], ps[:], kmin_sb[:, n * 512:(n + 1) * 512],
                                op=mybir.AluOpType.min)
                            nc.vector.tensor_scalar_min(
                                sm[:], sm[:], qmin_sb[:, jt:jt + 1])
                            nc.sync.dma_start(sc_d[b, rows, n * 512:(n + 1) * 512],
                                              sm[:])
                            nc.vector.reduce_max(
                                stt[:, n:n + 1], sm[:],
                                axis=mybir.AxisListType.X, negate=True)
                        # -max over the full row; exp both halves with row-sums
                        nc.vector.tensor_tensor(
                            stt[:, 2:3], stt[:, 0:1], stt[:, 1:2],
                            op=mybir.AluOpType.min)
                        for n in range(2):
                            nc.scalar.activation(
                                ew_sb[:, h * Lk + n * 512:h * Lk + (n + 1) * 512],
                                sm_n[n][:],
                                mybir.ActivationFunctionType.Exp,
                                bias=stt[:, 2:3], scale=1.0,
                                accum_out=stt[:, 3 + n:4 + n])
                        nc.vector.tensor_tensor(
                            stt[:, 5:6], stt[:, 3:4], stt[:, 4:5],
                            op=mybir.AluOpType.add)
                        nc.vector.reciprocal(stt[:, 6:7], stt[:, 5:6])
                        nc.vector.tensor_scalar_mul(
                            ew_sb[:, h * Lk:(h + 1) * Lk],
                            ew_sb[:, h * Lk:(h + 1) * Lk],
                            stt[:, 6:7])
                        # transpose W tiles: [128q, 128k] -> [128k, 128q]
                        for kt in range(8):
                            pt = psT.tile([128, 128], BF16, tag="psT")
                            nc.tensor.transpose(
                                pt[:],
                                ew_sb[:, h * Lk + kt * 128:h * Lk + (kt + 1) * 128],
                                ident[:])
                            nc.vector.tensor_copy(
                                wt_sb[:, kt * MQ + h * 128:kt * MQ + (h + 1) * 128],
                                pt[:])

                    # ---- stage 3: mixT = context.T @ W.T ------------------
                    mixT_sb = blk.tile([128, 8 * MQ], BF16, tag="qm")
                    for dt in range(8):
                        pm = psA.tile([128, MQ], F32, tag="psA")
                        for kt in range(8):
                            nc.tensor.matmul(
                                pm[:],
                                cn_sb[:, kt * D + dt * 128:kt * D + (dt + 1) * 128],
                                wt_sb[:, kt * MQ:(kt + 1) * MQ],
                                start=(kt == 0), stop=(kt == 7))
                        nc.vector.tensor_copy(mixT_sb[:, dt * MQ:(dt + 1) * MQ], pm[:])

                    # ---- stage 4: out = tanh([mix, q] @ W_out.T) ----------
                    for h in range(2):
                        rows = slice(q0 + h * 128, q0 + (h + 1) * 128)
                        for n in range(2):
                            po = psO.tile([128, 512], F32, tag="psO")
                            for dt in range(8):
                                nc.tensor.matmul(
                                    po[:],
                                    mixT_sb[:, dt * MQ + h * 128:dt * MQ + (h + 1) * 128],
                                    woT_sb[:, dt * D + n * 512:dt * D + (n + 1) * 512],
                                    start=(dt == 0), stop=False)
                            for et in range(8):
                                nc.tensor.matmul(
                                    po[:],
                                    qTb_sb[:, et * MQ + h * 128:et * MQ + (h + 1) * 128],
                                    woT_sb[:, (8 + et) * D + n * 512:(8 + et) * D + (n + 1) * 512],
                                    start=False, stop=(et == 7))
                            ot = ot_pool.tile([128, 512], F32, tag="ot")
                            nc.scalar.activation(
                                ot[:], po[:], mybir.ActivationFunctionType.Tanh)
                            nc.sync.dma_start(out_d[b, rows, n * 512:(n + 1) * 512],
                                              ot[:])

    nc.compile()
    return nc


def _get_program():
    if "nc" not in _program_cache:
        _program_cache["nc"] = _build_program()
    return _program_cache["nc"]


def kernel(query, context, query_lengths, context_lengths, W_in, W_out):
    nc = _get_program()

    qT = np.ascontiguousarray(query.transpose(0, 2, 1), dtype=np.float32)
    cT = np.ascontiguousarray(context.transpose(0, 2, 1), dtype=np.float32)
    cn = np.ascontiguousarray(context, dtype=np.float32).astype(np.float16)
    winT = np.ascontiguousarray(W_in.T, dtype=np.float32)
    woT = np.ascontiguousarray(W_out.T, dtype=np.float32).astype(np.float16)

    k_idx = np.arange(Lk, dtype=np.int64)
    q_idx = np.arange(Lq, dtype=np.int64)
    kmin = np.where(k_idx[None, :] < context_lengths[:, None].astype(np.int64),
                    np.float32(BIG), np.float32(NEG)).astype(np.float32)  # [B, Lk]
    qmin = np.where(q_idx[None, :] < query_lengths[:, None].astype(np.int64),
                    np.float32(BIG), np.float32(NEG)).astype(np.float32)  # [B, Lq]
    kmin_rep = np.ascontiguousarray(
        np.broadcast_to(kmin[:, None, :], (B, 128, Lk)), dtype=np.float32)
    qmin_til = np.ascontiguousarray(
        qmin.reshape(B, Lq // 128, 128).transpose(0, 2, 1), dtype=np.float32)

    in_maps = []
    for c in range(N_CORES):
        s = slice(c * BPC, (c + 1) * BPC)
        in_maps.append({
            "qT": qT[s], "cT": cT[s], "cn": cn[s],
            "winT": winT, "woT": woT,
            "kmin": kmin_rep[s], "qmin": qmin_til[s],
        })

    res = bass_utils.run_bass_kernel_spmd(nc, in_maps, core_ids=list(range(N_CORES)))
    _program_cache["last_result"] = res

    out = np.concatenate([res.results[c]["out"] for c in range(N_CORES)], axis=0)
    scores = np.concatenate([res.results[c]["sc"] for c in range(N_CORES)], axis=0)
    return out, scores
